# revision 5
# baseline (speedup 1.0000x reference)
"""Bass/Trainium2 kernel for nn_LIDARStateCost (retrieval_knn).

Math: for each query point xt[n], gather its K=20 nearest dataset points,
fit plane z = a*x + b*y + c via normal equations (A w = b with A = D^T D,
b = D^T z, D = [x, y, 1]), project xt onto the plane, and return
  cost = ||proj - xt||^2 + exp(proj_z) + boundary(x) + boundary(y).

Closed form used on device (per query):
  stats: Sxx Sxy Syy Sx Sy Sxz Syz Sz (sums over the K neighbors)
  adjugate of A = [[Sxx Sxy Sx],[Sxy Syy Sy],[Sx Sy K]] and det(A);
  num_i = adj(A) @ [Sxz Syz Sz]  (= w_i * det)
  P   = x*num0 + y*num1 + num2 - z*det   (= (pn + d) * det)
  Q   = num0^2 + num1^2 + det^2          (= nn * det^2)
  closeness = P^2 / Q
  proj_z    = z + det*P/Q
  cost = closeness + exp(proj_z)
       + sigmoid(10x-50) + 1 - sigmoid(10x+50)
       + sigmoid(10y-50) + 1 - sigmoid(10y+50)

Sharding: data-parallel over queries; 8 cores, 131072 queries each;
dataset (24 MB) replicated in each core's DRAM. Within a core, query
q_local = p*nt + t lives on SBUF partition p, column t (nt = 1024).

KNN gather: per-partition-single-index indirect DMA (the only form the
DMA_INDIRECT ucode resolves deterministically on TRN2 — multi-index
offset APs race at drain time). One instruction gathers 128 rows (one
per partition); K*nt instructions per core. idx is converted to int32
on the host (indices < 2^21, lossless), halving index streaming and
removing the on-device int64-word extraction.
"""
import numpy as np

import concourse.bacc as bacc
import concourse.bass as bass
import concourse.mybir as mybir
from concourse.tile import TileContext
from concourse.bass_utils import run_bass_kernel_spmd

N_PTS = 1048576
M_PTS = 2097152
K = 20
NCORES = 8
NS = N_PTS // NCORES      # queries per core
NT = NS // 128            # columns per partition (1024)

F32 = mybir.dt.float32
I32 = mybir.dt.int32


def build(nt=NT, b=16, m=M_PTS, ch=512):
    """Build the per-core SPMD kernel. b = columns per gather tile,
    ch = columns per solve chunk."""
    assert nt % b == 0 and nt % ch == 0
    nc = bacc.Bacc("TRN2", target_bir_lowering=False, debug=False,
                   num_devices=NCORES)
    ds = nc.dram_tensor("dataset", [m, 3], F32, kind="ExternalInput")
    xtd = nc.dram_tensor("xt", [128, nt, 3], F32, kind="ExternalInput")
    idxd = nc.dram_tensor("idx", [128, nt, K], I32, kind="ExternalInput")
    outd = nc.dram_tensor("out", [128, nt], F32, kind="ExternalOutput")

    TT = mybir.AluOpType
    AF = mybir.ActivationFunctionType

    with TileContext(nc) as tc:
        with (
            tc.tile_pool(name="persist", bufs=1) as pp,
            tc.tile_pool(name="idxp", bufs=3) as idxp,
            tc.tile_pool(name="gp", bufs=3) as gp,
            tc.tile_pool(name="prodp", bufs=3) as prodp,
            tc.tile_pool(name="solvep", bufs=1) as sp,
        ):
            # persistent per-core state
            xtb = pp.tile([128, nt, 3], F32, tag="xtb")
            Sxx = pp.tile([128, nt], F32, tag="sxx")
            Sxy = pp.tile([128, nt], F32, tag="sxy")
            Syy = pp.tile([128, nt], F32, tag="syy")
            Sxz = pp.tile([128, nt], F32, tag="sxz")
            Syz = pp.tile([128, nt], F32, tag="syz")
            S4 = pp.tile([128, nt, 3], F32, tag="s4")     # (Sx, Sy, Sz)
            ob = pp.tile([128, nt], F32, tag="ob")

            nc.sync.dma_start(out=xtb[:], in_=xtd[:])

            def solve_chunk(c0):
                """plane fit + projection + cost for columns [c0, c0+ch),
                then stream the finished output chunk out."""
                cs = slice(c0, c0 + ch)
                vxx, vxy, vyy = Sxx[:, cs], Sxy[:, cs], Syy[:, cs]
                vxz, vyz = Sxz[:, cs], Syz[:, cs]
                merge = "p t c -> p (t c)"
                vx = S4[:, cs, 0:1].rearrange(merge)
                vy = S4[:, cs, 1:2].rearrange(merge)
                vz = S4[:, cs, 2:3].rearrange(merge)
                xq = xtb[:, cs, 0:1].rearrange(merge)
                yq = xtb[:, cs, 1:2].rearrange(merge)
                zq = xtb[:, cs, 2:3].rearrange(merge)

                def T(tag):
                    return sp.tile([128, ch], F32, tag=tag, name=tag)

                t1, t2 = T("t1"), T("t2")
                c00, c01, c02 = T("c00"), T("c01"), T("c02")
                c11, c12, c22 = T("c11"), T("c12"), T("c22")
                det = T("det")
                n0, n1, n2 = T("n0"), T("n1"), T("n2")

                def cof(out, pa, pb, ma, mb):
                    # out = pa*pb - ma*mb
                    nc.vector.tensor_tensor(out=t1[:], in0=pa, in1=pb,
                                            op=TT.mult)
                    nc.vector.tensor_tensor(out=t2[:], in0=ma, in1=mb,
                                            op=TT.mult)
                    nc.vector.tensor_tensor(out=out, in0=t1[:], in1=t2[:],
                                            op=TT.subtract)

                kf = float(K)
                # c00 = Syy*K - Sy*Sy
                nc.vector.tensor_scalar_mul(out=t1[:], in0=vyy, scalar1=kf)
                nc.vector.tensor_tensor(out=t2[:], in0=vy, in1=vy, op=TT.mult)
                nc.vector.tensor_tensor(out=c00[:], in0=t1[:], in1=t2[:],
                                        op=TT.subtract)
                # c01 = Sx*Sy - Sxy*K
                nc.vector.tensor_tensor(out=t1[:], in0=vx, in1=vy, op=TT.mult)
                nc.vector.tensor_scalar_mul(out=t2[:], in0=vxy, scalar1=kf)
                nc.vector.tensor_tensor(out=c01[:], in0=t1[:], in1=t2[:],
                                        op=TT.subtract)
                cof(c02[:], vxy, vy, vyy, vx)      # c02 = Sxy*Sy - Syy*Sx
                # c11 = Sxx*K - Sx*Sx
                nc.vector.tensor_scalar_mul(out=t1[:], in0=vxx, scalar1=kf)
                nc.vector.tensor_tensor(out=t2[:], in0=vx, in1=vx, op=TT.mult)
                nc.vector.tensor_tensor(out=c11[:], in0=t1[:], in1=t2[:],
                                        op=TT.subtract)
                cof(c12[:], vxy, vx, vxx, vy)      # c12 = Sxy*Sx - Sxx*Sy
                cof(c22[:], vxx, vyy, vxy, vxy)    # c22 = Sxx*Syy - Sxy^2

                def dot3(out, a1, b1, a2, b2, a3, b3):
                    # out = a1*b1 + a2*b2 + a3*b3
                    nc.vector.tensor_tensor(out=out, in0=a1, in1=b1,
                                            op=TT.mult)
                    nc.vector.tensor_tensor(out=t1[:], in0=a2, in1=b2,
                                            op=TT.mult)
                    nc.vector.tensor_tensor(out=out, in0=out, in1=t1[:],
                                            op=TT.add)
                    nc.vector.tensor_tensor(out=t1[:], in0=a3, in1=b3,
                                            op=TT.mult)
                    nc.vector.tensor_tensor(out=out, in0=out, in1=t1[:],
                                            op=TT.add)

                dot3(det[:], vxx, c00[:], vxy, c01[:], vx, c02[:])
                dot3(n0[:], c00[:], vxz, c01[:], vyz, c02[:], vz)
                dot3(n1[:], c01[:], vxz, c11[:], vyz, c12[:], vz)
                dot3(n2[:], c02[:], vxz, c12[:], vyz, c22[:], vz)

                # P = x*n0 + y*n1 + n2 - z*det
                P, Q, rq = T("P"), T("Q"), T("rq")
                nc.vector.tensor_tensor(out=P[:], in0=xq, in1=n0[:], op=TT.mult)
                nc.vector.tensor_tensor(out=t1[:], in0=yq, in1=n1[:], op=TT.mult)
                nc.vector.tensor_tensor(out=P[:], in0=P[:], in1=t1[:], op=TT.add)
                nc.vector.tensor_tensor(out=P[:], in0=P[:], in1=n2[:], op=TT.add)
                nc.vector.tensor_tensor(out=t1[:], in0=zq, in1=det[:], op=TT.mult)
                nc.vector.tensor_tensor(out=P[:], in0=P[:], in1=t1[:], op=TT.subtract)

                dot3(Q[:], n0[:], n0[:], n1[:], n1[:], det[:], det[:])
                nc.vector.reciprocal(out=rq[:], in_=Q[:])

                prq, clos, tdet = T("prq"), T("clos"), T("tdet")
                nc.vector.tensor_tensor(out=prq[:], in0=P[:], in1=rq[:], op=TT.mult)
                nc.vector.tensor_tensor(out=clos[:], in0=prq[:], in1=P[:], op=TT.mult)
                nc.vector.tensor_tensor(out=tdet[:], in0=prq[:], in1=det[:], op=TT.mult)
                # proj_z = z + tdet ; height = exp(proj_z)
                zpt, h = T("zpt"), T("h")
                nc.vector.tensor_tensor(out=zpt[:], in0=zq, in1=tdet[:], op=TT.add)
                nc.scalar.activation(out=h[:], in_=zpt[:], func=AF.Exp)

                s1, s2, s3, s4v = T("s1"), T("s2"), T("s3"), T("s4v")
                for sdst, src, bias in ((s1, xq, -50.0), (s2, xq, 50.0),
                                        (s3, yq, -50.0), (s4v, yq, 50.0)):
                    nc.vector.tensor_scalar(out=sdst[:], in0=src,
                                            scalar1=10.0, scalar2=bias,
                                            op0=TT.mult, op1=TT.add)
                    nc.scalar.activation(out=sdst[:], in_=sdst[:],
                                         func=AF.Sigmoid)

                res = T("res")
                nc.vector.tensor_tensor(out=res[:], in0=clos[:], in1=h[:], op=TT.add)
                nc.vector.tensor_tensor(out=t1[:], in0=s1[:], in1=s3[:], op=TT.add)
                nc.vector.tensor_tensor(out=res[:], in0=res[:], in1=t1[:], op=TT.add)
                nc.vector.tensor_tensor(out=t1[:], in0=s2[:], in1=s4v[:], op=TT.add)
                nc.vector.tensor_tensor(out=res[:], in0=res[:], in1=t1[:], op=TT.subtract)
                nc.vector.tensor_scalar_add(out=ob[:, cs], in0=res[:], scalar1=2.0)
                # stream this finished chunk out (hides the output tail)
                nc.sync.dma_start(out=outd[:, cs], in_=ob[:, cs])

            done_cols = 0
            for t0 in range(0, nt, b):
                it = idxp.tile([128, b, K], I32, tag="it")
                nc.sync.dma_start(out=it[:], in_=idxd[:, t0:t0 + b, :])

                g = gp.tile([128, b, K, 3], F32, tag="g")
                for bb in range(b):
                    for k in range(K):
                        nc.gpsimd.indirect_dma_start(
                            out=g[:, bb, k, :],
                            out_offset=None,
                            in_=ds[:],
                            in_offset=bass.IndirectOffsetOnAxis(
                                ap=it[:, bb, k:k + 1], axis=0),
                        )

                gx = g[:, :, :, 0:1]
                gy = g[:, :, :, 1:2]
                gz = g[:, :, :, 2:3]
                pr = prodp.tile([128, b, K, 1], F32, tag="pr")
                prods = [
                    (gx, gx, Sxx), (gx, gy, Sxy), (gy, gy, Syy),
                    (gx, gz, Sxz), (gy, gz, Syz),
                ]
                for a, c, dest in prods:
                    nc.vector.tensor_tensor(out=pr[:], in0=a, in1=c,
                                            op=TT.mult)
                    nc.vector.tensor_reduce(
                        out=dest[:, t0:t0 + b],
                        in_=pr[:].rearrange("p t k one -> p t (k one)"),
                        axis=mybir.AxisListType.X, op=TT.add)
                # coordinate sums (Sx, Sy, Sz) in one strided reduce
                nc.vector.tensor_reduce(
                    out=S4[:, t0:t0 + b, :],
                    in_=g[:].rearrange("p t k c -> p t c k"),
                    axis=mybir.AxisListType.X, op=TT.add)

                # interleave solve chunks as their stats complete
                while done_cols + ch <= t0 + b:
                    solve_chunk(done_cols)
                    done_cols += ch

            while done_cols + ch <= nt:
                solve_chunk(done_cols)
                done_cols += ch

    nc.compile()
    return nc


_NC_CACHE = {}


def _get_nc(**kw):
    key = tuple(sorted(kw.items()))
    if key not in _NC_CACHE:
        _NC_CACHE[key] = build(**kw)
    return _NC_CACHE[key]


def kernel(xt, dataset, idx):
    xt = np.asarray(xt, dtype=np.float32)
    dataset = np.asarray(dataset, dtype=np.float32)
    idx32 = np.asarray(idx).astype(np.int32)  # values < 2^21, lossless

    nc = _get_nc()
    in_maps = []
    for c in range(NCORES):
        s = slice(c * NS, (c + 1) * NS)
        in_maps.append({
            "dataset": dataset,
            "xt": np.ascontiguousarray(xt[s]).reshape(128, NT, 3),
            "idx": np.ascontiguousarray(idx32[s]).reshape(128, NT, K),
        })
    res = run_bass_kernel_spmd(nc, in_maps, list(range(NCORES)), trace=False)
    out = np.empty(N_PTS, np.float32)
    for c in range(NCORES):
        out[c * NS:(c + 1) * NS] = res.results[c]["out"].ravel()
    return out


# revision 8
# speedup vs baseline: 1.0076x; 1.0076x over previous
"""Bass/Trainium2 kernel for nn_LIDARStateCost (retrieval_knn).

Math: for each query point xt[n], gather its K=20 nearest dataset points,
fit plane z = a*x + b*y + c via normal equations (A w = b with A = D^T D,
b = D^T z, D = [x, y, 1]), project xt onto the plane, and return
  cost = ||proj - xt||^2 + exp(proj_z) + boundary(x) + boundary(y).

Closed form used on device (per query):
  stats: Sxx Sxy Syy Sx Sy Sxz Syz Sz (sums over the K neighbors)
  adjugate of A = [[Sxx Sxy Sx],[Sxy Syy Sy],[Sx Sy K]] and det(A);
  num_i = adj(A) @ [Sxz Syz Sz]  (= w_i * det)
  P   = x*num0 + y*num1 + num2 - z*det   (= (pn + d) * det)
  Q   = num0^2 + num1^2 + det^2          (= nn * det^2)
  closeness = P^2 / Q
  proj_z    = z + det*P/Q
  cost = closeness + exp(proj_z)
       + sigmoid(10x-50) + 1 - sigmoid(10x+50)
       + sigmoid(10y-50) + 1 - sigmoid(10y+50)

Sharding: data-parallel over queries; 8 cores, 131072 queries each;
dataset (24 MB) replicated in each core's DRAM. Within a core, query
q_local = p*nt + t lives on SBUF partition p, column t (nt = 1024).

KNN gather: per-partition-single-index indirect DMA (the only form the
DMA_INDIRECT ucode resolves deterministically on TRN2 — multi-index
offset APs race at drain time). One instruction gathers 128 rows (one
per partition); K*nt instructions per core. idx is converted to int32
on the host (indices < 2^21, lossless), halving index streaming and
removing the on-device int64-word extraction.
"""
import numpy as np

import concourse.bacc as bacc
import concourse.bass as bass
import concourse.mybir as mybir
from concourse.tile import TileContext
from concourse.bass_utils import run_bass_kernel_spmd

N_PTS = 1048576
M_PTS = 2097152
K = 20
NCORES = 8
NS = N_PTS // NCORES      # queries per core
NT = NS // 128            # columns per partition (1024)

F32 = mybir.dt.float32
I32 = mybir.dt.int32


def build(nt=NT, b=16, m=M_PTS, ch=256):
    """Build the per-core SPMD kernel. b = columns per gather tile,
    ch = columns per solve chunk."""
    assert nt % b == 0 and nt % ch == 0
    nc = bacc.Bacc("TRN2", target_bir_lowering=False, debug=False,
                   num_devices=NCORES)
    ds = nc.dram_tensor("dataset", [m, 3], F32, kind="ExternalInput")
    xtd = nc.dram_tensor("xt", [128, nt, 3], F32, kind="ExternalInput")
    idxd = nc.dram_tensor("idx", [128, nt, K], I32, kind="ExternalInput")
    outd = nc.dram_tensor("out", [128, nt], F32, kind="ExternalOutput")

    TT = mybir.AluOpType
    AF = mybir.ActivationFunctionType

    with TileContext(nc) as tc:
        with (
            tc.tile_pool(name="persist", bufs=1) as pp,
            tc.tile_pool(name="idxp", bufs=3) as idxp,
            tc.tile_pool(name="gp", bufs=3) as gp,
            tc.tile_pool(name="prodp", bufs=3) as prodp,
            tc.tile_pool(name="solvep", bufs=1) as sp,
        ):
            # persistent per-core state
            xtb = pp.tile([128, nt, 3], F32, tag="xtb")
            Sxx = pp.tile([128, nt], F32, tag="sxx")
            Sxy = pp.tile([128, nt], F32, tag="sxy")
            Syy = pp.tile([128, nt], F32, tag="syy")
            Sxz = pp.tile([128, nt], F32, tag="sxz")
            Syz = pp.tile([128, nt], F32, tag="syz")
            S4 = pp.tile([128, nt, 3], F32, tag="s4")     # (Sx, Sy, Sz)
            ob = pp.tile([128, nt], F32, tag="ob")

            def solve_chunk(c0):
                """plane fit + projection + cost for columns [c0, c0+ch),
                then stream the finished output chunk out."""
                cs = slice(c0, c0 + ch)
                vxx, vxy, vyy = Sxx[:, cs], Sxy[:, cs], Syy[:, cs]
                vxz, vyz = Sxz[:, cs], Syz[:, cs]
                merge = "p t c -> p (t c)"
                vx = S4[:, cs, 0:1].rearrange(merge)
                vy = S4[:, cs, 1:2].rearrange(merge)
                vz = S4[:, cs, 2:3].rearrange(merge)
                xq = xtb[:, cs, 0:1].rearrange(merge)
                yq = xtb[:, cs, 1:2].rearrange(merge)
                zq = xtb[:, cs, 2:3].rearrange(merge)

                def T(tag):
                    return sp.tile([128, ch], F32, tag=tag, name=tag)

                t1, t2 = T("t1"), T("t2")
                c00, c01, c02 = T("c00"), T("c01"), T("c02")
                c11, c12, c22 = T("c11"), T("c12"), T("c22")
                det = T("det")
                n0, n1, n2 = T("n0"), T("n1"), T("n2")

                def cof(out, pa, pb, ma, mb):
                    # out = pa*pb - ma*mb
                    nc.vector.tensor_tensor(out=t1[:], in0=pa, in1=pb,
                                            op=TT.mult)
                    nc.vector.tensor_tensor(out=t2[:], in0=ma, in1=mb,
                                            op=TT.mult)
                    nc.vector.tensor_tensor(out=out, in0=t1[:], in1=t2[:],
                                            op=TT.subtract)

                kf = float(K)
                # c00 = Syy*K - Sy*Sy
                nc.vector.tensor_scalar_mul(out=t1[:], in0=vyy, scalar1=kf)
                nc.vector.tensor_tensor(out=t2[:], in0=vy, in1=vy, op=TT.mult)
                nc.vector.tensor_tensor(out=c00[:], in0=t1[:], in1=t2[:],
                                        op=TT.subtract)
                # c01 = Sx*Sy - Sxy*K
                nc.vector.tensor_tensor(out=t1[:], in0=vx, in1=vy, op=TT.mult)
                nc.vector.tensor_scalar_mul(out=t2[:], in0=vxy, scalar1=kf)
                nc.vector.tensor_tensor(out=c01[:], in0=t1[:], in1=t2[:],
                                        op=TT.subtract)
                cof(c02[:], vxy, vy, vyy, vx)      # c02 = Sxy*Sy - Syy*Sx
                # c11 = Sxx*K - Sx*Sx
                nc.vector.tensor_scalar_mul(out=t1[:], in0=vxx, scalar1=kf)
                nc.vector.tensor_tensor(out=t2[:], in0=vx, in1=vx, op=TT.mult)
                nc.vector.tensor_tensor(out=c11[:], in0=t1[:], in1=t2[:],
                                        op=TT.subtract)
                cof(c12[:], vxy, vx, vxx, vy)      # c12 = Sxy*Sx - Sxx*Sy
                cof(c22[:], vxx, vyy, vxy, vxy)    # c22 = Sxx*Syy - Sxy^2

                def dot3(out, a1, b1, a2, b2, a3, b3):
                    # out = a1*b1 + a2*b2 + a3*b3
                    nc.vector.tensor_tensor(out=out, in0=a1, in1=b1,
                                            op=TT.mult)
                    nc.vector.tensor_tensor(out=t1[:], in0=a2, in1=b2,
                                            op=TT.mult)
                    nc.vector.tensor_tensor(out=out, in0=out, in1=t1[:],
                                            op=TT.add)
                    nc.vector.tensor_tensor(out=t1[:], in0=a3, in1=b3,
                                            op=TT.mult)
                    nc.vector.tensor_tensor(out=out, in0=out, in1=t1[:],
                                            op=TT.add)

                dot3(det[:], vxx, c00[:], vxy, c01[:], vx, c02[:])
                dot3(n0[:], c00[:], vxz, c01[:], vyz, c02[:], vz)
                dot3(n1[:], c01[:], vxz, c11[:], vyz, c12[:], vz)
                dot3(n2[:], c02[:], vxz, c12[:], vyz, c22[:], vz)

                # P = x*n0 + y*n1 + n2 - z*det
                P, Q, rq = T("P"), T("Q"), T("rq")
                nc.vector.tensor_tensor(out=P[:], in0=xq, in1=n0[:], op=TT.mult)
                nc.vector.tensor_tensor(out=t1[:], in0=yq, in1=n1[:], op=TT.mult)
                nc.vector.tensor_tensor(out=P[:], in0=P[:], in1=t1[:], op=TT.add)
                nc.vector.tensor_tensor(out=P[:], in0=P[:], in1=n2[:], op=TT.add)
                nc.vector.tensor_tensor(out=t1[:], in0=zq, in1=det[:], op=TT.mult)
                nc.vector.tensor_tensor(out=P[:], in0=P[:], in1=t1[:], op=TT.subtract)

                dot3(Q[:], n0[:], n0[:], n1[:], n1[:], det[:], det[:])
                nc.vector.reciprocal(out=rq[:], in_=Q[:])

                prq, clos, tdet = T("prq"), T("clos"), T("tdet")
                nc.vector.tensor_tensor(out=prq[:], in0=P[:], in1=rq[:], op=TT.mult)
                nc.vector.tensor_tensor(out=clos[:], in0=prq[:], in1=P[:], op=TT.mult)
                nc.vector.tensor_tensor(out=tdet[:], in0=prq[:], in1=det[:], op=TT.mult)
                # proj_z = z + tdet ; height = exp(proj_z)
                zpt, h = T("zpt"), T("h")
                nc.vector.tensor_tensor(out=zpt[:], in0=zq, in1=tdet[:], op=TT.add)
                nc.scalar.activation(out=h[:], in_=zpt[:], func=AF.Exp)

                s1, s2, s3, s4v = T("s1"), T("s2"), T("s3"), T("s4v")
                for sdst, src, bias in ((s1, xq, -50.0), (s2, xq, 50.0),
                                        (s3, yq, -50.0), (s4v, yq, 50.0)):
                    nc.vector.tensor_scalar(out=sdst[:], in0=src,
                                            scalar1=10.0, scalar2=bias,
                                            op0=TT.mult, op1=TT.add)
                    nc.scalar.activation(out=sdst[:], in_=sdst[:],
                                         func=AF.Sigmoid)

                res = T("res")
                nc.vector.tensor_tensor(out=res[:], in0=clos[:], in1=h[:], op=TT.add)
                nc.vector.tensor_tensor(out=t1[:], in0=s1[:], in1=s3[:], op=TT.add)
                nc.vector.tensor_tensor(out=res[:], in0=res[:], in1=t1[:], op=TT.add)
                nc.vector.tensor_tensor(out=t1[:], in0=s2[:], in1=s4v[:], op=TT.add)
                nc.vector.tensor_tensor(out=res[:], in0=res[:], in1=t1[:], op=TT.subtract)
                nc.vector.tensor_scalar_add(out=ob[:, cs], in0=res[:], scalar1=2.0)
                # stream this finished chunk out (hides the output tail)
                nc.sync.dma_start(out=outd[:, cs], in_=ob[:, cs])

            done_cols = 0
            for t0 in range(0, nt, b):
                it = idxp.tile([128, b, K], I32, tag="it")
                nc.sync.dma_start(out=it[:], in_=idxd[:, t0:t0 + b, :])
                if t0 == 0:
                    # after the first idx tile so gathers start immediately;
                    # only needed by the first solve chunk much later
                    nc.sync.dma_start(out=xtb[:], in_=xtd[:])

                g = gp.tile([128, b, K, 3], F32, tag="g")
                for bb in range(b):
                    for k in range(K):
                        nc.gpsimd.indirect_dma_start(
                            out=g[:, bb, k, :],
                            out_offset=None,
                            in_=ds[:],
                            in_offset=bass.IndirectOffsetOnAxis(
                                ap=it[:, bb, k:k + 1], axis=0),
                        )

                gx = g[:, :, :, 0:1]
                gy = g[:, :, :, 1:2]
                gz = g[:, :, :, 2:3]
                pr = prodp.tile([128, b, K, 1], F32, tag="pr")
                prods = [
                    (gx, gx, Sxx), (gx, gy, Sxy), (gy, gy, Syy),
                    (gx, gz, Sxz), (gy, gz, Syz),
                ]
                for a, c, dest in prods:
                    nc.vector.tensor_tensor(out=pr[:], in0=a, in1=c,
                                            op=TT.mult)
                    nc.vector.tensor_reduce(
                        out=dest[:, t0:t0 + b],
                        in_=pr[:].rearrange("p t k one -> p t (k one)"),
                        axis=mybir.AxisListType.X, op=TT.add)
                # coordinate sums (Sx, Sy, Sz) in one strided reduce
                nc.vector.tensor_reduce(
                    out=S4[:, t0:t0 + b, :],
                    in_=g[:].rearrange("p t k c -> p t c k"),
                    axis=mybir.AxisListType.X, op=TT.add)

                # interleave solve chunks as their stats complete
                while done_cols + ch <= t0 + b:
                    solve_chunk(done_cols)
                    done_cols += ch

            while done_cols + ch <= nt:
                solve_chunk(done_cols)
                done_cols += ch

    nc.compile()
    return nc


_NC_CACHE = {}


def _get_nc(**kw):
    key = tuple(sorted(kw.items()))
    if key not in _NC_CACHE:
        _NC_CACHE[key] = build(**kw)
    return _NC_CACHE[key]


def kernel(xt, dataset, idx):
    xt = np.asarray(xt, dtype=np.float32)
    dataset = np.asarray(dataset, dtype=np.float32)
    idx32 = np.asarray(idx).astype(np.int32)  # values < 2^21, lossless

    nc = _get_nc()
    in_maps = []
    for c in range(NCORES):
        s = slice(c * NS, (c + 1) * NS)
        in_maps.append({
            "dataset": dataset,
            "xt": np.ascontiguousarray(xt[s]).reshape(128, NT, 3),
            "idx": np.ascontiguousarray(idx32[s]).reshape(128, NT, K),
        })
    res = run_bass_kernel_spmd(nc, in_maps, list(range(NCORES)), trace=False)
    out = np.empty(N_PTS, np.float32)
    for c in range(NCORES):
        out[c * NS:(c + 1) * NS] = res.results[c]["out"].ravel()
    return out
